# revision 1
# baseline (speedup 1.0000x reference)
"""Grouped GRU cell (nn_GRUCell) on 8 Trainium2 NeuronCores.

Problem shape: B=1024, I=256 groups, D=128.
  r   = sigmoid(X[:,i,None]*W_r[i] + hg @ U_r[i] + b_r[i])
  z   = sigmoid(X[:,i,None]*W_z[i] + hg @ U_z[i] + b_z[i])
  h~  = tanh   (X[:,i,None]*W_h[i] + (r*hg) @ U_h[i] + b_h[i])
  h'  = z*hg + (1-z)*h~
Outputs: (h' , h~), both [B, I*D].

Sharding: groups are fully independent -> 32 groups per core, no collectives.

Device layout: everything transposed, tiles are [D(part)=128, B(free)=1024]
per group.  The PE contracts over the partition dim, so h must be presented
with D on partitions; the host pre-permutes h to [group, D, B] (and permutes
the outputs back).  GEMMs use lhsT = U_g (natural [k, d_out]) as stationary
and hT as moving operand (N=512 chunks, float32r for 1 cyc/row).  The
X*W + b term is accumulated into the same PSUM banks with a K=2 rank-1
matmul: lhsT = [W_g ; b_g] (2x128), rhs = [X[:,i] ; ones] (2xN).
Sigmoid/Tanh run on ScalarE directly from PSUM; the GRU blend runs on
VectorE with the (hg - h~) subtraction offloaded to GpSimd.
"""

import os
from contextlib import ExitStack

import numpy as np

import concourse.bass as bass
import concourse.tile as tile
from concourse import bacc, mybir
from concourse.bass_utils import run_bass_kernel_spmd

B = 1024
I = 256
D = 128
NCORES = 8
GPC = I // NCORES  # 32 groups per core
NCHUNK = 2  # B is processed in 512-wide moving chunks
CHUNK = B // NCHUNK  # 512

# float32r: full-rate (1 cycle/row) fp32 matmul mode on the PE.
MM_DT = mybir.dt.float32r

_PROGRAM = None  # cached (nc)


def _build_program():
    nc = bacc.Bacc(
        "TRN2",
        target_bir_lowering=False,
        debug=False,
        enable_asserts=False,
    )

    hT_d = nc.dram_tensor("hT", [GPC, D, B], MM_DT, kind="ExternalInput").ap()
    # U packed on host as [k=128, (g, gate, d)] so the DMA is fully contiguous.
    u_d = nc.dram_tensor("ucat", [D, GPC * 3 * D], MM_DT, kind="ExternalInput").ap()
    # Per group: plane0 = [W_r|W_z|W_h|X_row], plane1 = [b_r|b_z|b_h|ones]
    wx_d = nc.dram_tensor("wx", [2, GPC, 3 * D + B], MM_DT, kind="ExternalInput").ap()
    hnT_d = nc.dram_tensor("hnT", [GPC, D, B], mybir.dt.float32, kind="ExternalOutput").ap()
    htT_d = nc.dram_tensor("htT", [GPC, D, B], mybir.dt.float32, kind="ExternalOutput").ap()

    sig = mybir.ActivationFunctionType.Sigmoid
    tanh = mybir.ActivationFunctionType.Tanh

    with tile.TileContext(nc) as tc, ExitStack() as ctx:
        u_pool = ctx.enter_context(tc.tile_pool(name="u", bufs=1))
        hT_pool = ctx.enter_context(tc.tile_pool(name="hT", bufs=4))
        wx_pool = ctx.enter_context(tc.tile_pool(name="wx", bufs=4))
        ps_pool = ctx.enter_context(tc.tile_pool(name="ps", bufs=4, space="PSUM"))
        act_pool = ctx.enter_context(tc.tile_pool(name="act", bufs=4))
        mid_pool = ctx.enter_context(tc.tile_pool(name="mid", bufs=4))
        out_pool = ctx.enter_context(tc.tile_pool(name="out", bufs=3))

        u_sb = u_pool.tile([D, GPC * 3 * D], MM_DT)
        nc.sync.dma_start(u_sb[:], u_d[:])

        def stage1(g):
            hT = hT_pool.tile([D, B], MM_DT, tag="hT", name=f"hT{g}")
            nc.sync.dma_start(hT[:], hT_d[g])
            wx = wx_pool.tile([2, 3 * D + B], MM_DT, tag="wx", name=f"wx{g}")
            nc.sync.dma_start(wx[:], wx_d[:, g, :])

            def gemm(psum, gate, moving):
                u_g = u_sb[:, (g * 3 + gate) * D : (g * 3 + gate + 1) * D]
                wb_g = wx[:, gate * D : (gate + 1) * D]
                for c in range(NCHUNK):
                    sl = slice(c * CHUNK, (c + 1) * CHUNK)
                    nc.tensor.matmul(
                        psum[:, sl], lhsT=u_g, rhs=moving[:, sl],
                        start=True, stop=False,
                    )
                for c in range(NCHUNK):
                    sl = slice(c * CHUNK, (c + 1) * CHUNK)
                    x2 = wx[:, 3 * D + c * CHUNK : 3 * D + (c + 1) * CHUNK]
                    nc.tensor.matmul(
                        psum[:, sl], lhsT=wb_g, rhs=x2,
                        start=False, stop=True,
                    )

            pr = ps_pool.tile([D, B], mybir.dt.float32, tag="ps", name=f"pr{g}")
            gemm(pr, 0, hT)
            pz = ps_pool.tile([D, B], mybir.dt.float32, tag="ps", name=f"pz{g}")
            gemm(pz, 1, hT)

            r = act_pool.tile([D, B], mybir.dt.float32, tag="r", name=f"r{g}")
            nc.scalar.activation(r[:], pr[:], sig)
            z = act_pool.tile([D, B], mybir.dt.float32, tag="z", name=f"z{g}")
            nc.scalar.activation(z[:], pz[:], sig)

            rh = mid_pool.tile([D, B], MM_DT, tag="rh", name=f"rh{g}")
            nc.vector.tensor_mul(rh[:], r[:], hT[:].bitcast(mybir.dt.float32))
            return dict(g=g, hT=hT, wx=wx, z=z, rh=rh, gemm=gemm)

        def stage2(s):
            g = s["g"]
            ph = ps_pool.tile([D, B], mybir.dt.float32, tag="ps", name=f"ph{g}")
            s["gemm"](ph, 2, s["rh"])
            ht = out_pool.tile([D, B], mybir.dt.float32, tag="ht", name=f"ht{g}")
            nc.scalar.activation(ht[:], ph[:], tanh)
            dd = mid_pool.tile([D, B], mybir.dt.float32, tag="dd", name=f"dd{g}")
            nc.vector.tensor_sub(dd[:], s["hT"][:].bitcast(mybir.dt.float32), ht[:])
            tz = mid_pool.tile([D, B], mybir.dt.float32, tag="tz", name=f"tz{g}")
            nc.vector.tensor_mul(tz[:], s["z"][:], dd[:])
            hn = out_pool.tile([D, B], mybir.dt.float32, tag="hn", name=f"hn{g}")
            nc.vector.tensor_add(hn[:], tz[:], ht[:])
            nc.gpsimd.dma_start(htT_d[g], ht[:])
            nc.gpsimd.dma_start(hnT_d[g], hn[:])

        DELAY = 2
        pend = []
        for g in range(GPC):
            pend.append(stage1(g))
            if len(pend) > DELAY:
                stage2(pend.pop(0))
        while pend:
            stage2(pend.pop(0))

    nc.compile()
    return nc


def _get_program():
    global _PROGRAM
    if _PROGRAM is None:
        _PROGRAM = _build_program()
    return _PROGRAM


LAST_EXEC_TIME_NS = None
LAST_RESULTS = None


def kernel(X, h, W_r, W_z, W_h, U_r, U_z, U_h, b_r, b_z, b_h):
    global LAST_EXEC_TIME_NS, LAST_RESULTS
    X = np.asarray(X, dtype=np.float32)
    h = np.asarray(h, dtype=np.float32)
    W = np.stack([np.asarray(W_r), np.asarray(W_z), np.asarray(W_h)], axis=1).astype(
        np.float32
    )  # [I, 3, 1, D]
    U = np.stack([np.asarray(U_r), np.asarray(U_z), np.asarray(U_h)], axis=1).astype(
        np.float32
    )  # [I, 3, D, D]
    bb = np.stack([np.asarray(b_r), np.asarray(b_z), np.asarray(b_h)], axis=1).astype(
        np.float32
    )  # [I, 3, D]

    # [I, D, B] transposed h
    hT = np.ascontiguousarray(h.reshape(B, I, D).transpose(1, 2, 0))
    XT = np.ascontiguousarray(X.T)  # [I, B]
    ones = np.ones((GPC, 1, B), dtype=np.float32)

    in_maps = []
    for c in range(NCORES):
        sl = slice(c * GPC, (c + 1) * GPC)
        u_core = U[sl]  # [GPC, 3, D(k), D(d)]
        u_sb = np.ascontiguousarray(
            u_core.transpose(2, 0, 1, 3).reshape(D, GPC * 3 * D)
        )
        w_core = W[sl, :, 0, :].reshape(GPC, 3 * D)  # [GPC, 3D]
        b_core = bb[sl].reshape(GPC, 3 * D)
        plane0 = np.concatenate([w_core, XT[sl]], axis=1)  # [GPC, 3D+B]
        plane1 = np.concatenate([b_core, ones[:, 0, :]], axis=1)
        wx = np.ascontiguousarray(np.stack([plane0, plane1], axis=0))  # [2, GPC, 3D+B]
        in_maps.append(
            {
                "hT": np.ascontiguousarray(hT[sl]),
                "ucat": u_sb,
                "wx": wx,
            }
        )

    nc = _get_program()
    trace = bool(int(os.environ.get("KERNEL_TRACE", "0")))
    res = run_bass_kernel_spmd(nc, in_maps, core_ids=list(range(NCORES)), trace=trace)
    LAST_EXEC_TIME_NS = res.exec_time_ns
    LAST_RESULTS = res

    hnT = np.concatenate([res.results[c]["hnT"] for c in range(NCORES)], axis=0)
    htT = np.concatenate([res.results[c]["htT"] for c in range(NCORES)], axis=0)
    h_new = np.ascontiguousarray(hnT.transpose(2, 0, 1)).reshape(B, I * D)
    h_tilde = np.ascontiguousarray(htT.transpose(2, 0, 1)).reshape(B, I * D)
    return h_new, h_tilde



# revision 2
# speedup vs baseline: 1.0739x; 1.0739x over previous
"""Grouped GRU cell (nn_GRUCell) on 8 Trainium2 NeuronCores.

Problem shape: B=1024, I=256 groups, D=128.
  r   = sigmoid(X[:,i,None]*W_r[i] + hg @ U_r[i] + b_r[i])
  z   = sigmoid(X[:,i,None]*W_z[i] + hg @ U_z[i] + b_z[i])
  h~  = tanh   (X[:,i,None]*W_h[i] + (r*hg) @ U_h[i] + b_h[i])
  h'  = z*hg + (1-z)*h~
Outputs: (h', h~), both [B, I*D].

Sharding: groups are fully independent -> 32 groups per core, no collectives.

Device layout: tiles are [D(part)=128, B(free)=1024] per group; host
pre-permutes h to [group, D, B] and permutes outputs back.  All DMA
payloads are fp16 (tolerance is 2e-2; fp16 keeps us ~1e-3), which halves
HBM traffic and enables the DVE 2x packed mode.

Engine plan per group (steady state is ScalarE-bound at ~3.4us/group):
  PE     : 3 gate GEMMs as 2x N=512 fp16 matmuls each, plus the X*W+b
           term accumulated into the same PSUM bank as a K=2 rank-1
           matmul (lhsT=[W;b], rhs=[X;1]) -- matmul cost is N-driven, so
           K=2 costs the same as K=128, but it keeps the work off the
           busier engines.
  Scalar : every activation is tanh: sigmoid(a) = 0.5*tanh(a/2)+0.5, so
           r/z use ACT(scale=0.5) and the affine fixup rides for free in
           the DVE ops' scalar slots; U_h is pre-halved on the host so
           the h~ GEMM consumes (tanh_r+1)*h = 2*r*h directly.
  DVE    : rh2 = (tanh_r+1)*h; d = h-h~; e = (tanh_z+1)*d; hn = 0.5*e+h~
           (= z*h + (1-z)*h~). All fp16 -> 2x packed mode.
  Pool   : output DMA triggers only (gpsimd elementwise is 0.42x eff).
PSUM: pr(2 banks) + pz(2) + ph(2x2 double-buffered) = 8 banks.
"""

import os
from contextlib import ExitStack

import numpy as np

import concourse.bass as bass
import concourse.tile as tile
from concourse import bacc, mybir
from concourse.bass_utils import run_bass_kernel_spmd

B = 1024
I = 256
D = 128
NCORES = 8
GPC = I // NCORES  # 32 groups per core
NCHUNK = 2  # B is processed in 512-wide chunks (one PSUM bank each)
CHUNK = B // NCHUNK  # 512
WXW = 3 * D + B  # per-group width of the wx plane

F16 = mybir.dt.float16
F32 = mybir.dt.float32

_PROGRAM = None


def _build_program():
    nc = bacc.Bacc(
        "TRN2",
        target_bir_lowering=False,
        debug=False,
        enable_asserts=False,
    )

    hT_d = nc.dram_tensor("hT", [GPC, D, B], F16, kind="ExternalInput").ap()
    # U packed on host as [k=128, (g, gate, d)]; U_h pre-scaled by 0.5.
    u_d = nc.dram_tensor("ucat", [D, GPC * 3 * D], F16, kind="ExternalInput").ap()
    # Per group: plane0 = [W_r|W_z|W_h|X_row], plane1 = [b_r|b_z|b_h|ones]
    wx_d = nc.dram_tensor("wx", [2, GPC * WXW], F16, kind="ExternalInput").ap()
    hnT_d = nc.dram_tensor("hnT", [GPC, D, B], F16, kind="ExternalOutput").ap()
    htT_d = nc.dram_tensor("htT", [GPC, D, B], F16, kind="ExternalOutput").ap()

    tanh = mybir.ActivationFunctionType.Tanh
    Alu = mybir.AluOpType

    with tile.TileContext(nc) as tc, ExitStack() as ctx:
        u_pool = ctx.enter_context(tc.tile_pool(name="u", bufs=1))
        wx_pool = ctx.enter_context(tc.tile_pool(name="wx", bufs=1))
        hT_pool = ctx.enter_context(tc.tile_pool(name="hT", bufs=4))
        pr_pool = ctx.enter_context(tc.tile_pool(name="pr", bufs=1, space="PSUM"))
        pz_pool = ctx.enter_context(tc.tile_pool(name="pz", bufs=1, space="PSUM"))
        ph_pool = ctx.enter_context(tc.tile_pool(name="ph", bufs=2, space="PSUM"))
        act_pool = ctx.enter_context(tc.tile_pool(name="act", bufs=4))
        mid_pool = ctx.enter_context(tc.tile_pool(name="mid", bufs=6))
        out_pool = ctx.enter_context(tc.tile_pool(name="out", bufs=4))

        wx_sb = wx_pool.tile([2, GPC * WXW], F16)
        nc.sync.dma_start(wx_sb[:], wx_d[:])
        u_sb = u_pool.tile([D, GPC * 3 * D], F16)
        # Chunked so group 0's weights land quickly and MMs start early.
        UCH = GPC // 4 * 3 * D
        for c in range(4):
            nc.sync.dma_start(u_sb[:, c * UCH : (c + 1) * UCH], u_d[:, c * UCH : (c + 1) * UCH])

        def gemm(psum, g, gate, moving):
            u_g = u_sb[:, (g * 3 + gate) * D : (g * 3 + gate + 1) * D]
            wb_g = wx_sb[:, g * WXW + gate * D : g * WXW + (gate + 1) * D]
            for c in range(NCHUNK):
                sl = slice(c * CHUNK, (c + 1) * CHUNK)
                nc.tensor.matmul(
                    psum[:, sl], lhsT=u_g, rhs=moving[:, sl],
                    start=True, stop=False,
                )
            for c in range(NCHUNK):
                sl = slice(c * CHUNK, (c + 1) * CHUNK)
                x1 = wx_sb[:, g * WXW + 3 * D + c * CHUNK : g * WXW + 3 * D + (c + 1) * CHUNK]
                nc.tensor.matmul(
                    psum[:, sl], lhsT=wb_g, rhs=x1,
                    start=False, stop=True,
                )

        def stageA(g):
            hT = hT_pool.tile([D, B], F16, tag="hT", name=f"hT{g}")
            nc.sync.dma_start(hT[:], hT_d[g])

            pr = pr_pool.tile([D, B], F32, tag="pr", name=f"pr{g}")
            gemm(pr, g, 0, hT)
            rt = act_pool.tile([D, B], F16, tag="rt", name=f"rt{g}")
            nc.scalar.activation(rt[:], pr[:], tanh, scale=0.5)
            rh2 = mid_pool.tile([D, B], F16, tag="rh2", name=f"rh2{g}")
            # (tanh_r + 1) * h = 2*r*h; U_h is pre-halved to compensate.
            nc.vector.scalar_tensor_tensor(
                rh2[:], rt[:], 1.0, hT[:], op0=Alu.add, op1=Alu.mult
            )

            pz = pz_pool.tile([D, B], F32, tag="pz", name=f"pz{g}")
            gemm(pz, g, 1, hT)
            zt = act_pool.tile([D, B], F16, tag="zt", name=f"zt{g}")
            nc.scalar.activation(zt[:], pz[:], tanh, scale=0.5)
            return dict(g=g, hT=hT, zt=zt, rh2=rh2)

        def stageB(s):
            g = s["g"]
            ph = ph_pool.tile([D, B], F32, tag="ph", name=f"ph{g}")
            gemm(ph, g, 2, s["rh2"])
            ht = out_pool.tile([D, B], F16, tag="ht", name=f"ht{g}")
            nc.scalar.activation(ht[:], ph[:], tanh)
            d = mid_pool.tile([D, B], F16, tag="d", name=f"d{g}")
            nc.vector.tensor_sub(d[:], s["hT"][:], ht[:])
            e = mid_pool.tile([D, B], F16, tag="e", name=f"e{g}")
            nc.vector.scalar_tensor_tensor(
                e[:], s["zt"][:], 1.0, d[:], op0=Alu.add, op1=Alu.mult
            )
            hn = out_pool.tile([D, B], F16, tag="hn", name=f"hn{g}")
            # 0.5*e + h~  =  z*(h-h~) + h~  =  z*h + (1-z)*h~
            nc.vector.scalar_tensor_tensor(
                hn[:], e[:], 0.5, ht[:], op0=Alu.mult, op1=Alu.add
            )
            nc.gpsimd.dma_start(htT_d[g], ht[:])
            nc.gpsimd.dma_start(hnT_d[g], hn[:])

        pend = None
        for g in range(GPC):
            cur = stageA(g)
            if pend is not None:
                stageB(pend)
            pend = cur
        stageB(pend)

    nc.compile()
    return nc


def _get_program():
    global _PROGRAM
    if _PROGRAM is None:
        _PROGRAM = _build_program()
    return _PROGRAM


LAST_EXEC_TIME_NS = None
LAST_RESULTS = None


def kernel(X, h, W_r, W_z, W_h, U_r, U_z, U_h, b_r, b_z, b_h):
    global LAST_EXEC_TIME_NS, LAST_RESULTS
    X = np.asarray(X, dtype=np.float32)
    h = np.asarray(h, dtype=np.float32)
    W = np.stack([np.asarray(W_r), np.asarray(W_z), np.asarray(W_h)], axis=1).astype(
        np.float32
    )  # [I, 3, 1, D]
    U = np.stack(
        [np.asarray(U_r), np.asarray(U_z), 0.5 * np.asarray(U_h)], axis=1
    ).astype(np.float32)  # [I, 3, D, D]  (U_h pre-halved)
    bb = np.stack([np.asarray(b_r), np.asarray(b_z), np.asarray(b_h)], axis=1).astype(
        np.float32
    )  # [I, 3, D]

    hT = np.ascontiguousarray(h.reshape(B, I, D).transpose(1, 2, 0)).astype(np.float16)
    XT = np.ascontiguousarray(X.T).astype(np.float16)  # [I, B]
    ones = np.ones((GPC, B), dtype=np.float16)

    in_maps = []
    for c in range(NCORES):
        sl = slice(c * GPC, (c + 1) * GPC)
        u_core = U[sl]  # [GPC, 3, D(k), D(d)]
        u_sb = np.ascontiguousarray(
            u_core.transpose(2, 0, 1, 3).reshape(D, GPC * 3 * D)
        ).astype(np.float16)
        w_core = W[sl, :, 0, :].reshape(GPC, 3 * D).astype(np.float16)
        b_core = bb[sl].reshape(GPC, 3 * D).astype(np.float16)
        plane0 = np.concatenate([w_core, XT[sl]], axis=1)  # [GPC, 3D+B]
        plane1 = np.concatenate([b_core, ones], axis=1)
        wx = np.ascontiguousarray(np.stack([plane0, plane1], axis=0)).reshape(
            2, GPC * WXW
        )
        in_maps.append(
            {
                "hT": np.ascontiguousarray(hT[sl]),
                "ucat": u_sb,
                "wx": wx,
            }
        )

    nc = _get_program()
    trace = bool(int(os.environ.get("KERNEL_TRACE", "0")))
    res = run_bass_kernel_spmd(nc, in_maps, core_ids=list(range(NCORES)), trace=trace)
    LAST_EXEC_TIME_NS = res.exec_time_ns
    LAST_RESULTS = res

    hnT = np.concatenate([res.results[c]["hnT"] for c in range(NCORES)], axis=0)
    htT = np.concatenate([res.results[c]["htT"] for c in range(NCORES)], axis=0)
    h_new = (
        np.ascontiguousarray(hnT.transpose(2, 0, 1)).reshape(B, I * D).astype(np.float32)
    )
    h_tilde = (
        np.ascontiguousarray(htT.transpose(2, 0, 1)).reshape(B, I * D).astype(np.float32)
    )
    return h_new, h_tilde
